# revision 24
# baseline (speedup 1.0000x reference)
"""Trainium2 Bass kernel for nn_JResCOPAttn (B=1, L=1024, D=128).

Reference computation:
    a   = x @ Wl.T + bl                        # [L, D]
    tm  = (a[:,None,:] * a[None,:,:]) @ Wlo.T + blo    # [L, L, D]  (never materialized!)
    tm *= (mask != 0)
    tx  = x @ Wl2.T + bl2                      # [L, D]
    y   = x + einsum('cad,ad->cd', tm, tx)
    out = LayerNorm(y) * gamma + beta

Algebraic restructuring used here (per output row c):
    y1[c,d] = sum_e act[c,e] * WloT[e,d] * S_c[e,d]  +  blo[d] * Z[c,d]
    S_c[e,d] = sum_a act[a,e] * (mask[c,a]*tx[a,d])      (8 accumulating matmuls)
    Z[c,d]   = sum_a mask[c,a] * tx[a,d]                 (one batch of matmuls)
This avoids materializing the 536MB tm tensor entirely.

Performance structure (bf16 everywhere hot; fp32 residual/LayerNorm):
  * The PE matmuls are 512 wide: for a quad of 4 c's the moving operand is
    the masked tx for all four, laid out [a, (d, c)] (d-major).  512-wide
    matmuls sustain full PE rate; 128-wide ones pay 2x overhead.
  * The mask-apply (the irreducible 16.8M-element-per-core intermediate) is
    split DVE (t 0-4, one broadcast mega-multiply) / GpSimd (t 5-6) /
    Scalar (t 7, per-c scale ops).  The [t, d, c] iteration order keeps the
    broadcast tx operand stride-0 on the last dim, which runs at full DVE
    rate under concurrency (the [t, c, d] order is 2.4x slower).
  * g4 = S .* WloT is one packed DVE multiply straight out of PSUM; the
    per-c matvec stationary reads it with a stride-4 access pattern.
  * The quad loop is software-pipelined (masks i / matmuls i-1 / finals i-2)
    so no engine queue head-of-line blocks a later stage.  The kernel runs at
    the chip's aggregate SBUF-access roofline (~2.1 rows/ns across engines).

Sharding: rows c are split across the 8 NeuronCores (128 rows each); x is
replicated so each core computes act/tx for all 1024 source rows locally.
"""

import os
import sys

for _p in ("/opt/trn_rl_repo", "/root/.axon_site/_ro/trn_rl_repo"):
    if os.path.isdir(_p) and _p not in sys.path:
        sys.path.insert(0, _p)

import numpy as np
import ml_dtypes

import concourse.bass as bass
import concourse.tile as tile
from concourse import bacc, mybir
from concourse.bass_utils import run_bass_kernel_spmd
from concourse.masks import make_identity

B, L, D = 1, 1024, 128
NCORES = 8
CB = L // NCORES          # c-rows per core = 128
T = L // 128              # a-tiles = 8
EPS = 1e-5
FP = mybir.dt.float32
BF = mybir.dt.bfloat16
QUAD = 4                  # c's per PSUM bank / per wide matmul

# per-quad mask-apply split: t-tiles assigned to each engine
DVE_TSL = (0, 5)          # DVE: one mega broadcast multiply over t in [0,5)
GP_TSL = (5, 7)           # GpSimd: one mega broadcast multiply over t in [5,7)
SC_T = (7,)               # Scalar: per-(c,t) activation-scale ops


def build_nc():
    nc = bacc.Bacc("TRN2", target_bir_lowering=False)

    # ---- I/O ----
    xT    = nc.dram_tensor("xT",    [128, L], BF, kind="ExternalInput")    # x^T bf16
    xTb   = nc.dram_tensor("xTb",   [128, CB], BF, kind="ExternalInput")   # this core's block of xT cols
    xrow  = nc.dram_tensor("xrow",  [CB, D], FP, kind="ExternalInput")     # this core's x rows (residual)
    mTb   = nc.dram_tensor("mTb",   [128, T, CB], BF, kind="ExternalInput")  # mTb[p,t,c] = mask[c0+c, t*128+p]
    mTf   = nc.dram_tensor("mTf",   [128, T, CB], FP, kind="ExternalInput")  # fp32 copy for scalar operands
    WlT   = nc.dram_tensor("WlT",   [128, 128], BF, kind="ExternalInput")  # Wl.T
    Wl2T  = nc.dram_tensor("Wl2T",  [128, 128], BF, kind="ExternalInput")  # Wl2.T
    Wlodc = nc.dram_tensor("Wlodc", [128, 128, QUAD], BF, kind="ExternalInput")  # WloT[e,d] replicated over c
    blrow = nc.dram_tensor("blrow", [1, 128], BF, kind="ExternalInput")    # bl as row (bias matmul)
    bl2row = nc.dram_tensor("bl2row", [1, 128], BF, kind="ExternalInput")
    bl    = nc.dram_tensor("bl",    [128, 1], FP, kind="ExternalInput")
    blo   = nc.dram_tensor("blo",   [128, 1], FP, kind="ExternalInput")
    gam   = nc.dram_tensor("gam",   [CB, D], FP, kind="ExternalInput")     # gamma broadcast to rows
    bet   = nc.dram_tensor("bet",   [CB, D], FP, kind="ExternalInput")
    out   = nc.dram_tensor("out",   [CB, D], FP, kind="ExternalOutput")

    Ident = mybir.ActivationFunctionType.Identity
    Sqrt = mybir.ActivationFunctionType.Sqrt

    with tile.TileContext(nc) as tc:
        with (
            tc.tile_pool(name="singles", bufs=1) as singles,
            tc.tile_pool(name="trps", bufs=2, space="PSUM") as trps,
            tc.tile_pool(name="setps", bufs=2, space="PSUM") as setps,
            tc.tile_pool(name="ma", bufs=4) as ma_pool,
            tc.tile_pool(name="g", bufs=3) as g_pool,
            tc.tile_pool(name="s4", bufs=3, space="PSUM") as s4_pool,
            tc.tile_pool(name="y1tp", bufs=1, space="PSUM") as y1t_pool,
        ):
            # ---- load constants / inputs ----
            sb_xT = singles.tile([128, L], BF)
            nc.sync.dma_start(sb_xT, xT[:, :])
            sb_xTb = singles.tile([128, CB], BF)
            nc.sync.dma_start(sb_xTb, xTb[:, :])
            sb_xrow = singles.tile([CB, D], FP)
            nc.sync.dma_start(sb_xrow, xrow[:, :])
            sb_mTb = singles.tile([128, T, CB], BF)
            nc.scalar.dma_start(sb_mTb, mTb[:, :, :])
            sb_mTf = singles.tile([128, T, CB], FP)
            nc.gpsimd.dma_start(sb_mTf, mTf[:, :, :])
            sb_WlT = singles.tile([128, 128], BF)
            nc.sync.dma_start(sb_WlT, WlT[:, :])
            sb_Wl2T = singles.tile([128, 128], BF)
            nc.sync.dma_start(sb_Wl2T, Wl2T[:, :])
            sb_Wlodc = singles.tile([128, 128, QUAD], BF)
            nc.gpsimd.dma_start(sb_Wlodc, Wlodc[:, :, :])
            sb_blrow = singles.tile([1, 128], BF)
            nc.sync.dma_start(sb_blrow, blrow[:, :])
            sb_bl2row = singles.tile([1, 128], BF)
            nc.sync.dma_start(sb_bl2row, bl2row[:, :])
            sb_bl = singles.tile([128, 1], FP)
            nc.sync.dma_start(sb_bl, bl[:, :])
            sb_blo = singles.tile([128, 1], FP)
            nc.sync.dma_start(sb_blo, blo[:, :])
            sb_gam = singles.tile([CB, D], FP)
            nc.sync.dma_start(sb_gam, gam[:, :])
            sb_bet = singles.tile([CB, D], FP)
            nc.sync.dma_start(sb_bet, bet[:, :])

            ones1 = singles.tile([1, 128], BF)
            nc.gpsimd.memset(ones1, 1.0)
            sb_eps = singles.tile([CB, 1], FP)
            nc.vector.memset(sb_eps, EPS)

            # ---- act/tx directly in natural [a, e] layout, bias via K=1 matmul ----
            act_nat = singles.tile([128, T, 128], BF)
            tx_nat = singles.tile([128, T, 128], BF)
            for t in range(T):
                sl = slice(t * 128, (t + 1) * 128)
                p1 = trps.tile([128, 128], FP, tag="tr")
                nc.tensor.matmul(p1, sb_xT[:, sl], sb_WlT, start=True, stop=False)
                nc.tensor.matmul(p1, ones1, sb_blrow, start=False, stop=True)
                nc.scalar.copy(act_nat[:, t, :], p1)
                p2 = trps.tile([128, 128], FP, tag="tr")
                nc.tensor.matmul(p2, sb_xT[:, sl], sb_Wl2T, start=True, stop=False)
                nc.tensor.matmul(p2, ones1, sb_bl2row, start=False, stop=True)
                nc.scalar.copy(tx_nat[:, t, :], p2)

            # actT restricted to this core's c-block (matvec moving operand)
            actTb = singles.tile([128, CB], BF)
            ps_b = setps.tile([128, CB], FP, tag="set_mm")
            nc.tensor.matmul(ps_b, sb_WlT, sb_xTb, start=True, stop=True)
            nc.scalar.activation(actTb, ps_b, Ident, bias=sb_bl, scale=1.0)

            # ---- ZT[d,c] = sum_a tx[a,d] * mask[c,a];  bloZT = blo * ZT ----
            zt_ps = setps.tile([128, CB], FP, tag="set_mm")
            for t in range(T):
                nc.tensor.matmul(
                    zt_ps, tx_nat[:, t, :], sb_mTb[:, t, :],
                    start=(t == 0), stop=(t == T - 1),
                )
            bloZT = singles.tile([128, CB], FP)
            nc.vector.tensor_scalar_mul(bloZT, zt_ps, sb_blo)

            # ---- main loop over this core's 128 output rows, 4 at a time ----
            # software pipelined: iteration i issues masks(i), matmuls(i-1),
            # g4+matvecs(i-2) so no engine queue blocks on a later stage.
            y1t_ps = y1t_pool.tile([128, CB], FP)  # Y1^T columns, [d, c]
            d0, d1 = DVE_TSL
            g0, g1 = GP_TSL
            NQ = CB // QUAD
            ma_t = [None] * NQ
            s4_t = [None] * NQ

            def stage_masks(cq):
                c0 = cq * QUAD
                # ma[p, t, d, j] = tx[p, t, d] * m[p, t, c0+j]   ([t,d,c] order)
                ma = ma_pool.tile([128, T, 128, QUAD], BF, tag="ma")
                ma_t[cq] = ma
                nc.vector.tensor_mul(
                    ma[:, d0:d1, :, :],
                    tx_nat[:, d0:d1, :].unsqueeze(3).broadcast_to((128, d1 - d0, 128, QUAD)),
                    sb_mTb[:, d0:d1, c0:c0 + QUAD].unsqueeze(2).broadcast_to((128, d1 - d0, 128, QUAD)),
                )
                nc.gpsimd.tensor_mul(
                    ma[:, g0:g1, :, :],
                    tx_nat[:, g0:g1, :].unsqueeze(3).broadcast_to((128, g1 - g0, 128, QUAD)),
                    sb_mTb[:, g0:g1, c0:c0 + QUAD].unsqueeze(2).broadcast_to((128, g1 - g0, 128, QUAD)),
                )
                for t in SC_T:
                    for j in range(QUAD):
                        nc.scalar.mul(
                            ma[:, t, :, j], tx_nat[:, t, :], sb_mTf[:, t, c0 + j:c0 + j + 1]
                        )

            def stage_matmuls(cq):
                # S for the quad: 8 wide accumulating matmuls, out [e, (d, c)]
                s4 = s4_pool.tile([128, 128, QUAD], FP)
                s4_t[cq] = s4
                ma = ma_t[cq]
                for t in range(T):
                    nc.tensor.matmul(
                        s4[:, :, :], act_nat[:, t, :], ma[:, t, :, :],
                        start=(t == 0), stop=(t == T - 1),
                    )

            def stage_final(cq):
                c0 = cq * QUAD
                s4 = s4_t[cq]
                # Scalar engine (which has slack) drains PSUM to bf16; DVE
                # multiplies by WloT fully packed at 2x rate.
                sg4 = g_pool.tile([128, 128, QUAD], BF, tag="sg4")
                nc.scalar.copy(sg4, s4)
                g4 = g_pool.tile([128, 128, QUAD], BF, tag="g4")
                nc.vector.tensor_mul(g4, sg4, sb_Wlodc)
                for j in range(QUAD):
                    c = c0 + j
                    nc.tensor.matmul(
                        y1t_ps[:, c:c + 1], g4[:, :, j], actTb[:, c:c + 1],
                        start=True, stop=True,
                    )

            for i in range(NQ + 2):
                if i < NQ:
                    stage_masks(i)
                if 1 <= i < NQ + 1:
                    stage_matmuls(i - 1)
                if i >= 2:
                    stage_final(i - 2)

            # ---- combine, transpose back, residual, LayerNorm ----
            ident = singles.tile([128, 128], FP)
            make_identity(nc, ident)

            yt_sb = singles.tile([128, CB], FP)
            nc.vector.tensor_add(yt_sb, y1t_ps, bloZT)           # [d, c]
            y_ps = trps.tile([128, 128], FP, tag="tr")
            nc.tensor.transpose(y_ps, yt_sb, ident)              # [c, d]
            y_sb = singles.tile([CB, D], FP)
            nc.vector.tensor_add(y_sb, y_ps, sb_xrow)            # + x residual

            stats = singles.tile([CB, nc.vector.BN_STATS_DIM], FP)
            nc.vector.bn_stats(stats, y_sb)
            mv = singles.tile([CB, 2], FP)
            nc.vector.bn_aggr(mv, stats)
            nc.vector.tensor_scalar_sub(y_sb, y_sb, mv[:, 0:1])  # y - mean
            sd = singles.tile([CB, 1], FP)
            nc.scalar.activation(sd, mv[:, 1:2], Sqrt, bias=sb_eps, scale=1.0)
            rstd = singles.tile([CB, 1], FP)
            nc.vector.reciprocal(rstd, sd)
            nc.vector.tensor_scalar_mul(y_sb, y_sb, rstd)
            nc.vector.tensor_mul(y_sb, y_sb, sb_gam)
            nc.vector.tensor_add(y_sb, y_sb, sb_bet)

            nc.sync.dma_start(out[:, :], y_sb)

    return nc


_NC_CACHE = None


def _get_nc():
    global _NC_CACHE
    if _NC_CACHE is None:
        _NC_CACHE = build_nc()
        _NC_CACHE.finalize()
    return _NC_CACHE


def _prepare_in_maps(x, mask, Wl, bl, Wlo, blo, Wl2, bl2, gamma, beta):
    f32 = np.float32
    bf16 = ml_dtypes.bfloat16
    x0 = np.ascontiguousarray(np.asarray(x, f32)[0])          # [L, D]
    m = np.asarray(mask)[0].astype(f32)                       # [L, L] (c, a)
    xT = np.ascontiguousarray(x0.T)                           # [128, L]
    WlT = np.ascontiguousarray(np.asarray(Wl, f32).T)
    Wl2T = np.ascontiguousarray(np.asarray(Wl2, f32).T)
    WloT = np.ascontiguousarray(np.asarray(Wlo, f32).T)       # [e, d]
    Wlodc = np.ascontiguousarray(
        np.broadcast_to(WloT[:, :, None], (128, 128, QUAD))
    ).astype(bf16)
    bl_c = np.asarray(bl, f32).reshape(128, 1)
    blo_c = np.asarray(blo, f32).reshape(128, 1)
    blrow = np.asarray(bl, f32).reshape(1, 128).astype(bf16)
    bl2row = np.asarray(bl2, f32).reshape(1, 128).astype(bf16)
    gam_b = np.ascontiguousarray(np.broadcast_to(np.asarray(gamma, f32), (CB, D)))
    bet_b = np.ascontiguousarray(np.broadcast_to(np.asarray(beta, f32), (CB, D)))
    xT_bf = xT.astype(bf16)

    in_maps = []
    for k in range(NCORES):
        blk = slice(k * CB, (k + 1) * CB)
        mTk = m[blk, :].T.reshape(T, 128, CB).transpose(1, 0, 2)  # [p, t, c]
        mTk = np.ascontiguousarray(mTk)
        in_maps.append({
            "xT": xT_bf,
            "xTb": np.ascontiguousarray(xT_bf[:, blk]),
            "xrow": np.ascontiguousarray(x0[blk]),
            "mTb": mTk.astype(bf16),
            "mTf": mTk,
            "WlT": WlT.astype(bf16),
            "Wl2T": Wl2T.astype(bf16),
            "Wlodc": Wlodc,
            "blrow": blrow,
            "bl2row": bl2row,
            "bl": bl_c,
            "blo": blo_c,
            "gam": gam_b,
            "bet": bet_b,
        })
    return in_maps


def kernel(x, mask, Wl, bl, Wlo, blo, Wl2, bl2, gamma, beta):
    in_maps = _prepare_in_maps(x, mask, Wl, bl, Wlo, blo, Wl2, bl2, gamma, beta)
    res = run_bass_kernel_spmd(_get_nc(), in_maps, core_ids=list(range(NCORES)))
    y = np.concatenate([res.results[k]["out"] for k in range(NCORES)], axis=0)
    return y.reshape(B, L, D).astype(np.float32)


# revision 25
# speedup vs baseline: 1.0443x; 1.0443x over previous
"""Trainium2 Bass kernel for nn_JResCOPAttn (B=1, L=1024, D=128).

Reference computation:
    a   = x @ Wl.T + bl                        # [L, D]
    tm  = (a[:,None,:] * a[None,:,:]) @ Wlo.T + blo    # [L, L, D]  (never materialized!)
    tm *= (mask != 0)
    tx  = x @ Wl2.T + bl2                      # [L, D]
    y   = x + einsum('cad,ad->cd', tm, tx)
    out = LayerNorm(y) * gamma + beta

Algebraic restructuring used here (per output row c):
    y1[c,d] = sum_e act[c,e] * WloT[e,d] * S_c[e,d]  +  blo[d] * Z[c,d]
    S_c[e,d] = sum_a act[a,e] * (mask[c,a]*tx[a,d])      (8 accumulating matmuls)
    Z[c,d]   = sum_a mask[c,a] * tx[a,d]                 (one batch of matmuls)
This avoids materializing the 536MB tm tensor entirely.

Performance structure (bf16 everywhere hot; fp32 residual/LayerNorm):
  * The PE matmuls are 512 wide: for a quad of 4 c's the moving operand is
    the masked tx for all four, laid out [a, (d, c)] (d-major).  512-wide
    matmuls sustain full PE rate; 128-wide ones pay 2x overhead.
  * The mask-apply (the irreducible 16.8M-element-per-core intermediate) is
    split DVE (t 0-4, one broadcast mega-multiply) / GpSimd (t 5-6) /
    Scalar (t 7, per-c scale ops).  The [t, d, c] iteration order keeps the
    broadcast tx operand stride-0 on the last dim, which runs at full DVE
    rate under concurrency (the [t, c, d] order is 2.4x slower).
  * g4 = S .* WloT is one packed DVE multiply straight out of PSUM; the
    per-c matvec stationary reads it with a stride-4 access pattern.
  * The quad loop is software-pipelined (masks i / matmuls i-1 / finals i-2)
    so no engine queue head-of-line blocks a later stage.  The kernel runs at
    the chip's aggregate SBUF-access roofline (~2.1 rows/ns across engines).

Sharding: rows c are split across the 8 NeuronCores (128 rows each); x is
replicated so each core computes act/tx for all 1024 source rows locally.
"""

import os
import sys

for _p in ("/opt/trn_rl_repo", "/root/.axon_site/_ro/trn_rl_repo"):
    if os.path.isdir(_p) and _p not in sys.path:
        sys.path.insert(0, _p)

import numpy as np
import ml_dtypes

import concourse.bass as bass
import concourse.tile as tile
from concourse import bacc, mybir
from concourse.bass_utils import run_bass_kernel_spmd
from concourse.masks import make_identity

B, L, D = 1, 1024, 128
NCORES = 8
CB = L // NCORES          # c-rows per core = 128
T = L // 128              # a-tiles = 8
EPS = 1e-5
FP = mybir.dt.float32
BF = mybir.dt.bfloat16
QUAD = 4                  # c's per PSUM bank / per wide matmul

# per-quad mask-apply split: t-tiles assigned to each engine
DVE_TSL = (0, 5)          # DVE: one mega broadcast multiply over t in [0,5)
GP_TSL = (5, 7)           # GpSimd: one mega broadcast multiply over t in [5,7)
SC_T = (7,)               # Scalar: per-(c,t) activation-scale ops


def build_nc():
    nc = bacc.Bacc("TRN2", target_bir_lowering=False)

    # ---- I/O ----
    xT    = nc.dram_tensor("xT",    [128, L], BF, kind="ExternalInput")    # x^T bf16
    xTb   = nc.dram_tensor("xTb",   [128, CB], BF, kind="ExternalInput")   # this core's block of xT cols
    xrow  = nc.dram_tensor("xrow",  [CB, D], FP, kind="ExternalInput")     # this core's x rows (residual)
    mTb   = nc.dram_tensor("mTb",   [128, T, CB], BF, kind="ExternalInput")  # mTb[p,t,c] = mask[c0+c, t*128+p]
    mTf   = nc.dram_tensor("mTf",   [128, T, CB], FP, kind="ExternalInput")  # fp32 copy for scalar operands
    WlT   = nc.dram_tensor("WlT",   [128, 128], BF, kind="ExternalInput")  # Wl.T
    Wl2T  = nc.dram_tensor("Wl2T",  [128, 128], BF, kind="ExternalInput")  # Wl2.T
    Wlodc = nc.dram_tensor("Wlodc", [128, 128, QUAD], BF, kind="ExternalInput")  # WloT[e,d] replicated over c
    blrow = nc.dram_tensor("blrow", [1, 128], BF, kind="ExternalInput")    # bl as row (bias matmul)
    bl2row = nc.dram_tensor("bl2row", [1, 128], BF, kind="ExternalInput")
    bl    = nc.dram_tensor("bl",    [128, 1], FP, kind="ExternalInput")
    blo   = nc.dram_tensor("blo",   [128, 1], FP, kind="ExternalInput")
    gam   = nc.dram_tensor("gam",   [CB, D], FP, kind="ExternalInput")     # gamma broadcast to rows
    bet   = nc.dram_tensor("bet",   [CB, D], FP, kind="ExternalInput")
    out   = nc.dram_tensor("out",   [CB, D], FP, kind="ExternalOutput")

    Ident = mybir.ActivationFunctionType.Identity
    Sqrt = mybir.ActivationFunctionType.Sqrt

    with tile.TileContext(nc) as tc:
        with (
            tc.tile_pool(name="singles", bufs=1) as singles,
            tc.tile_pool(name="trps", bufs=2, space="PSUM") as trps,
            tc.tile_pool(name="setps", bufs=2, space="PSUM") as setps,
            tc.tile_pool(name="ma", bufs=4) as ma_pool,
            tc.tile_pool(name="g", bufs=2) as g_pool,
            tc.tile_pool(name="s4", bufs=3, space="PSUM") as s4_pool,
            tc.tile_pool(name="y1tp", bufs=1, space="PSUM") as y1t_pool,
        ):
            # ---- load constants / inputs ----
            sb_xT = singles.tile([128, L], BF)
            nc.sync.dma_start(sb_xT, xT[:, :])
            sb_xTb = singles.tile([128, CB], BF)
            nc.sync.dma_start(sb_xTb, xTb[:, :])
            sb_xrow = singles.tile([CB, D], FP)
            nc.sync.dma_start(sb_xrow, xrow[:, :])
            sb_mTb = singles.tile([128, T, CB], BF)
            nc.sync.dma_start(sb_mTb, mTb[:, :, :])
            sb_mTf = singles.tile([128, T, CB], FP)
            nc.sync.dma_start(sb_mTf, mTf[:, :, :])
            sb_WlT = singles.tile([128, 128], BF)
            nc.sync.dma_start(sb_WlT, WlT[:, :])
            sb_Wl2T = singles.tile([128, 128], BF)
            nc.sync.dma_start(sb_Wl2T, Wl2T[:, :])
            sb_Wlodc = singles.tile([128, 128, QUAD], BF)
            nc.sync.dma_start(sb_Wlodc, Wlodc[:, :, :])
            sb_blrow = singles.tile([1, 128], BF)
            nc.sync.dma_start(sb_blrow, blrow[:, :])
            sb_bl2row = singles.tile([1, 128], BF)
            nc.sync.dma_start(sb_bl2row, bl2row[:, :])
            sb_bl = singles.tile([128, 1], FP)
            nc.sync.dma_start(sb_bl, bl[:, :])
            sb_blo = singles.tile([128, 1], FP)
            nc.sync.dma_start(sb_blo, blo[:, :])
            sb_gam = singles.tile([CB, D], FP)
            nc.sync.dma_start(sb_gam, gam[:, :])
            sb_bet = singles.tile([CB, D], FP)
            nc.sync.dma_start(sb_bet, bet[:, :])

            ones1 = singles.tile([1, 128], BF)
            nc.gpsimd.memset(ones1, 1.0)
            sb_eps = singles.tile([CB, 1], FP)
            nc.vector.memset(sb_eps, EPS)

            # ---- act/tx directly in natural [a, e] layout, bias via K=1 matmul ----
            act_nat = singles.tile([128, T, 128], BF)
            tx_nat = singles.tile([128, T, 128], BF)
            for t in range(T):
                sl = slice(t * 128, (t + 1) * 128)
                p1 = trps.tile([128, 128], FP, tag="tr")
                nc.tensor.matmul(p1, sb_xT[:, sl], sb_WlT, start=True, stop=False)
                nc.tensor.matmul(p1, ones1, sb_blrow, start=False, stop=True)
                nc.scalar.copy(act_nat[:, t, :], p1)
                p2 = trps.tile([128, 128], FP, tag="tr")
                nc.tensor.matmul(p2, sb_xT[:, sl], sb_Wl2T, start=True, stop=False)
                nc.tensor.matmul(p2, ones1, sb_bl2row, start=False, stop=True)
                nc.scalar.copy(tx_nat[:, t, :], p2)

            # actT restricted to this core's c-block (matvec moving operand)
            actTb = singles.tile([128, CB], BF)
            ps_b = setps.tile([128, CB], FP, tag="set_mm")
            nc.tensor.matmul(ps_b, sb_WlT, sb_xTb, start=True, stop=True)
            nc.scalar.activation(actTb, ps_b, Ident, bias=sb_bl, scale=1.0)

            # ---- ZT[d,c] = sum_a tx[a,d] * mask[c,a];  bloZT = blo * ZT ----
            zt_ps = setps.tile([128, CB], FP, tag="set_mm")
            for t in range(T):
                nc.tensor.matmul(
                    zt_ps, tx_nat[:, t, :], sb_mTb[:, t, :],
                    start=(t == 0), stop=(t == T - 1),
                )
            bloZT = singles.tile([128, CB], FP)
            nc.vector.tensor_scalar_mul(bloZT, zt_ps, sb_blo)

            # ---- main loop over this core's 128 output rows, 4 at a time ----
            # software pipelined: iteration i issues masks(i), matmuls(i-1),
            # g4+matvecs(i-2) so no engine queue blocks on a later stage.
            y1t_ps = y1t_pool.tile([128, CB], FP)  # Y1^T columns, [d, c]
            d0, d1 = DVE_TSL
            g0, g1 = GP_TSL
            NQ = CB // QUAD
            ma_t = [None] * NQ
            s4_t = [None] * NQ

            def stage_masks(cq):
                c0 = cq * QUAD
                # ma[p, t, d, j] = tx[p, t, d] * m[p, t, c0+j]   ([t,d,c] order)
                ma = ma_pool.tile([128, T, 128, QUAD], BF, tag="ma")
                ma_t[cq] = ma
                nc.vector.tensor_mul(
                    ma[:, d0:d1, :, :],
                    tx_nat[:, d0:d1, :].unsqueeze(3).broadcast_to((128, d1 - d0, 128, QUAD)),
                    sb_mTb[:, d0:d1, c0:c0 + QUAD].unsqueeze(2).broadcast_to((128, d1 - d0, 128, QUAD)),
                )
                nc.gpsimd.tensor_mul(
                    ma[:, g0:g1, :, :],
                    tx_nat[:, g0:g1, :].unsqueeze(3).broadcast_to((128, g1 - g0, 128, QUAD)),
                    sb_mTb[:, g0:g1, c0:c0 + QUAD].unsqueeze(2).broadcast_to((128, g1 - g0, 128, QUAD)),
                )
                for t in SC_T:
                    for j in range(QUAD):
                        nc.scalar.mul(
                            ma[:, t, :, j], tx_nat[:, t, :], sb_mTf[:, t, c0 + j:c0 + j + 1]
                        )

            def stage_matmuls(cq):
                # S for the quad: 8 wide accumulating matmuls, out [e, (d, c)]
                s4 = s4_pool.tile([128, 128, QUAD], FP)
                s4_t[cq] = s4
                ma = ma_t[cq]
                for t in range(T):
                    nc.tensor.matmul(
                        s4[:, :, :], act_nat[:, t, :], ma[:, t, :, :],
                        start=(t == 0), stop=(t == T - 1),
                    )

            def stage_final(cq):
                c0 = cq * QUAD
                s4 = s4_t[cq]
                # g4[e, d, c] = S[e, d, c] * WloT[e, d]: one packed DVE op
                # straight out of PSUM; the matvec stationary reads stride-4.
                g4 = g_pool.tile([128, 128, QUAD], BF, tag="g4")
                nc.vector.tensor_mul(g4, s4, sb_Wlodc)
                for j in range(QUAD):
                    c = c0 + j
                    nc.tensor.matmul(
                        y1t_ps[:, c:c + 1], g4[:, :, j], actTb[:, c:c + 1],
                        start=True, stop=True,
                    )

            for i in range(NQ + 2):
                if i < NQ:
                    stage_masks(i)
                if 1 <= i < NQ + 1:
                    stage_matmuls(i - 1)
                if i >= 2:
                    stage_final(i - 2)

            # ---- combine, transpose back, residual, LayerNorm ----
            ident = singles.tile([128, 128], FP)
            make_identity(nc, ident)

            yt_sb = singles.tile([128, CB], FP)
            nc.vector.tensor_add(yt_sb, y1t_ps, bloZT)           # [d, c]
            y_ps = trps.tile([128, 128], FP, tag="tr")
            nc.tensor.transpose(y_ps, yt_sb, ident)              # [c, d]
            y_sb = singles.tile([CB, D], FP)
            nc.vector.tensor_add(y_sb, y_ps, sb_xrow)            # + x residual

            stats = singles.tile([CB, nc.vector.BN_STATS_DIM], FP)
            nc.vector.bn_stats(stats, y_sb)
            mv = singles.tile([CB, 2], FP)
            nc.vector.bn_aggr(mv, stats)
            nc.vector.tensor_scalar_sub(y_sb, y_sb, mv[:, 0:1])  # y - mean
            sd = singles.tile([CB, 1], FP)
            nc.scalar.activation(sd, mv[:, 1:2], Sqrt, bias=sb_eps, scale=1.0)
            rstd = singles.tile([CB, 1], FP)
            nc.vector.reciprocal(rstd, sd)
            nc.vector.tensor_scalar_mul(y_sb, y_sb, rstd)
            nc.vector.tensor_mul(y_sb, y_sb, sb_gam)
            nc.vector.tensor_add(y_sb, y_sb, sb_bet)

            nc.sync.dma_start(out[:, :], y_sb)

    return nc


_NC_CACHE = None


def _get_nc():
    global _NC_CACHE
    if _NC_CACHE is None:
        _NC_CACHE = build_nc()
        _NC_CACHE.finalize()
    return _NC_CACHE


def _prepare_in_maps(x, mask, Wl, bl, Wlo, blo, Wl2, bl2, gamma, beta):
    f32 = np.float32
    bf16 = ml_dtypes.bfloat16
    x0 = np.ascontiguousarray(np.asarray(x, f32)[0])          # [L, D]
    m = np.asarray(mask)[0].astype(f32)                       # [L, L] (c, a)
    xT = np.ascontiguousarray(x0.T)                           # [128, L]
    WlT = np.ascontiguousarray(np.asarray(Wl, f32).T)
    Wl2T = np.ascontiguousarray(np.asarray(Wl2, f32).T)
    WloT = np.ascontiguousarray(np.asarray(Wlo, f32).T)       # [e, d]
    Wlodc = np.ascontiguousarray(
        np.broadcast_to(WloT[:, :, None], (128, 128, QUAD))
    ).astype(bf16)
    bl_c = np.asarray(bl, f32).reshape(128, 1)
    blo_c = np.asarray(blo, f32).reshape(128, 1)
    blrow = np.asarray(bl, f32).reshape(1, 128).astype(bf16)
    bl2row = np.asarray(bl2, f32).reshape(1, 128).astype(bf16)
    gam_b = np.ascontiguousarray(np.broadcast_to(np.asarray(gamma, f32), (CB, D)))
    bet_b = np.ascontiguousarray(np.broadcast_to(np.asarray(beta, f32), (CB, D)))
    xT_bf = xT.astype(bf16)

    in_maps = []
    for k in range(NCORES):
        blk = slice(k * CB, (k + 1) * CB)
        mTk = m[blk, :].T.reshape(T, 128, CB).transpose(1, 0, 2)  # [p, t, c]
        mTk = np.ascontiguousarray(mTk)
        in_maps.append({
            "xT": xT_bf,
            "xTb": np.ascontiguousarray(xT_bf[:, blk]),
            "xrow": np.ascontiguousarray(x0[blk]),
            "mTb": mTk.astype(bf16),
            "mTf": mTk,
            "WlT": WlT.astype(bf16),
            "Wl2T": Wl2T.astype(bf16),
            "Wlodc": Wlodc,
            "blrow": blrow,
            "bl2row": bl2row,
            "bl": bl_c,
            "blo": blo_c,
            "gam": gam_b,
            "bet": bet_b,
        })
    return in_maps


def kernel(x, mask, Wl, bl, Wlo, blo, Wl2, bl2, gamma, beta):
    in_maps = _prepare_in_maps(x, mask, Wl, bl, Wlo, blo, Wl2, bl2, gamma, beta)
    res = run_bass_kernel_spmd(_get_nc(), in_maps, core_ids=list(range(NCORES)))
    y = np.concatenate([res.results[k]["out"] for k in range(NCORES)], axis=0)
    return y.reshape(B, L, D).astype(np.float32)


# revision 26
# speedup vs baseline: 1.0622x; 1.0171x over previous
"""Trainium2 Bass kernel for nn_JResCOPAttn (B=1, L=1024, D=128).

Reference computation:
    a   = x @ Wl.T + bl                        # [L, D]
    tm  = (a[:,None,:] * a[None,:,:]) @ Wlo.T + blo    # [L, L, D]  (never materialized!)
    tm *= (mask != 0)
    tx  = x @ Wl2.T + bl2                      # [L, D]
    y   = x + einsum('cad,ad->cd', tm, tx)
    out = LayerNorm(y) * gamma + beta

Algebraic restructuring used here (per output row c):
    y1[c,d] = sum_e act[c,e] * WloT[e,d] * S_c[e,d]  +  blo[d] * Z[c,d]
    S_c[e,d] = sum_a act[a,e] * (mask[c,a]*tx[a,d])      (8 accumulating matmuls)
    Z[c,d]   = sum_a mask[c,a] * tx[a,d]                 (one batch of matmuls)
This avoids materializing the 536MB tm tensor entirely.

Performance structure (bf16 everywhere hot; fp32 residual/LayerNorm):
  * The PE matmuls are 512 wide: for a quad of 4 c's the moving operand is
    the masked tx for all four, laid out [a, (d, c)] (d-major).  512-wide
    matmuls sustain full PE rate; 128-wide ones pay 2x overhead.
  * The mask-apply (the irreducible 16.8M-element-per-core intermediate) is
    split DVE (t 0-4, one broadcast mega-multiply) / GpSimd (t 5-6) /
    Scalar (t 7, per-c scale ops).  The [t, d, c] iteration order keeps the
    broadcast tx operand stride-0 on the last dim, which runs at full DVE
    rate under concurrency (the [t, c, d] order is 2.4x slower).
  * g4 = S .* WloT is one packed DVE multiply straight out of PSUM; the
    per-c matvec stationary reads it with a stride-4 access pattern.
  * The quad loop is software-pipelined (masks i / matmuls i-1 / finals i-2)
    so no engine queue head-of-line blocks a later stage.  The kernel runs at
    the chip's aggregate SBUF-access roofline (~2.1 rows/ns across engines).

Sharding: rows c are split across the 8 NeuronCores (128 rows each); x is
replicated so each core computes act/tx for all 1024 source rows locally.
"""

import os
import sys

for _p in ("/opt/trn_rl_repo", "/root/.axon_site/_ro/trn_rl_repo"):
    if os.path.isdir(_p) and _p not in sys.path:
        sys.path.insert(0, _p)

import numpy as np
import ml_dtypes

import concourse.bass as bass
import concourse.tile as tile
from concourse import bacc, mybir
from concourse.bass_utils import run_bass_kernel_spmd
from concourse.masks import make_identity

B, L, D = 1, 1024, 128
NCORES = 8
CB = L // NCORES          # c-rows per core = 128
T = L // 128              # a-tiles = 8
EPS = 1e-5
FP = mybir.dt.float32
BF = mybir.dt.bfloat16
QUAD = 4                  # c's per PSUM bank / per wide matmul

# per-quad mask-apply split: t-tiles assigned to each engine
DVE_TSL = (0, 5)          # DVE: one mega broadcast multiply over t in [0,5)
GP_TSL = (5, 7)           # GpSimd: one mega broadcast multiply over t in [5,7)
SC_T = (7,)               # Scalar: per-(c,t) activation-scale ops


def build_nc():
    nc = bacc.Bacc("TRN2", target_bir_lowering=False)

    # ---- I/O ----
    xT    = nc.dram_tensor("xT",    [128, L], BF, kind="ExternalInput")    # x^T bf16
    xTb   = nc.dram_tensor("xTb",   [128, CB], BF, kind="ExternalInput")   # this core's block of xT cols
    xrow  = nc.dram_tensor("xrow",  [CB, D], FP, kind="ExternalInput")     # this core's x rows (residual)
    mTb   = nc.dram_tensor("mTb",   [128, T, CB], BF, kind="ExternalInput")  # mTb[p,t,c] = mask[c0+c, t*128+p]
    mTf   = nc.dram_tensor("mTf",   [128, T, CB], FP, kind="ExternalInput")  # fp32 copy for scalar operands
    WlT   = nc.dram_tensor("WlT",   [128, 128], BF, kind="ExternalInput")  # Wl.T
    Wl2T  = nc.dram_tensor("Wl2T",  [128, 128], BF, kind="ExternalInput")  # Wl2.T
    Wlodc = nc.dram_tensor("Wlodc", [128, 128, QUAD], BF, kind="ExternalInput")  # WloT[e,d] replicated over c
    blrow = nc.dram_tensor("blrow", [1, 128], BF, kind="ExternalInput")    # bl as row (bias matmul)
    bl2row = nc.dram_tensor("bl2row", [1, 128], BF, kind="ExternalInput")
    bl    = nc.dram_tensor("bl",    [128, 1], FP, kind="ExternalInput")
    blo   = nc.dram_tensor("blo",   [128, 1], FP, kind="ExternalInput")
    gam   = nc.dram_tensor("gam",   [CB, D], FP, kind="ExternalInput")     # gamma broadcast to rows
    bet   = nc.dram_tensor("bet",   [CB, D], FP, kind="ExternalInput")
    out   = nc.dram_tensor("out",   [CB, D], FP, kind="ExternalOutput")

    Ident = mybir.ActivationFunctionType.Identity
    Sqrt = mybir.ActivationFunctionType.Sqrt

    with tile.TileContext(nc) as tc:
        with (
            tc.tile_pool(name="singles", bufs=1) as singles,
            tc.tile_pool(name="trps", bufs=2, space="PSUM") as trps,
            tc.tile_pool(name="setps", bufs=2, space="PSUM") as setps,
            tc.tile_pool(name="ma", bufs=4) as ma_pool,
            tc.tile_pool(name="g", bufs=2) as g_pool,
            tc.tile_pool(name="s4", bufs=3, space="PSUM") as s4_pool,
            tc.tile_pool(name="y1tp", bufs=1, space="PSUM") as y1t_pool,
        ):
            # ---- load constants / inputs ----
            # DMA issue order = criticality: prep weights first (unblocks the
            # act/tx matmuls ~2us in), then the bf16 mask (unblocks the quad
            # megas), then everything else.
            sb_xT = singles.tile([128, L], BF)
            sb_xTb = singles.tile([128, CB], BF)
            sb_xrow = singles.tile([CB, D], FP)
            sb_mTb = singles.tile([128, T, CB], BF)
            sb_mTf = singles.tile([128, T, CB], FP)
            sb_WlT = singles.tile([128, 128], BF)
            sb_Wl2T = singles.tile([128, 128], BF)
            sb_Wlodc = singles.tile([128, 128, QUAD], BF)
            sb_blrow = singles.tile([1, 128], BF)
            sb_bl2row = singles.tile([1, 128], BF)
            sb_bl = singles.tile([128, 1], FP)
            sb_blo = singles.tile([128, 1], FP)
            sb_gam = singles.tile([CB, D], FP)
            sb_bet = singles.tile([CB, D], FP)

            nc.sync.dma_start(sb_WlT, WlT[:, :])
            nc.sync.dma_start(sb_Wl2T, Wl2T[:, :])
            nc.sync.dma_start(sb_blrow, blrow[:, :])
            nc.sync.dma_start(sb_bl2row, bl2row[:, :])
            nc.sync.dma_start(sb_bl, bl[:, :])
            nc.sync.dma_start(sb_xT, xT[:, :])
            nc.sync.dma_start(sb_mTb, mTb[:, :, :])
            nc.sync.dma_start(sb_xTb, xTb[:, :])
            nc.sync.dma_start(sb_Wlodc, Wlodc[:, :, :])
            nc.sync.dma_start(sb_blo, blo[:, :])
            nc.sync.dma_start(sb_mTf, mTf[:, :, :])
            nc.sync.dma_start(sb_xrow, xrow[:, :])
            nc.sync.dma_start(sb_gam, gam[:, :])
            nc.sync.dma_start(sb_bet, bet[:, :])

            ones1 = singles.tile([1, 128], BF)
            nc.gpsimd.memset(ones1, 1.0)
            sb_eps = singles.tile([CB, 1], FP)
            nc.vector.memset(sb_eps, EPS)

            # ---- act/tx directly in natural [a, e] layout, bias via K=1 matmul ----
            act_nat = singles.tile([128, T, 128], BF)
            tx_nat = singles.tile([128, T, 128], BF)
            for t in range(T):
                sl = slice(t * 128, (t + 1) * 128)
                p1 = trps.tile([128, 128], FP, tag="tr")
                nc.tensor.matmul(p1, sb_xT[:, sl], sb_WlT, start=True, stop=False)
                nc.tensor.matmul(p1, ones1, sb_blrow, start=False, stop=True)
                nc.scalar.copy(act_nat[:, t, :], p1)
                p2 = trps.tile([128, 128], FP, tag="tr")
                nc.tensor.matmul(p2, sb_xT[:, sl], sb_Wl2T, start=True, stop=False)
                nc.tensor.matmul(p2, ones1, sb_bl2row, start=False, stop=True)
                nc.scalar.copy(tx_nat[:, t, :], p2)

            # actT restricted to this core's c-block (matvec moving operand)
            actTb = singles.tile([128, CB], BF)
            ps_b = setps.tile([128, CB], FP, tag="set_mm")
            nc.tensor.matmul(ps_b, sb_WlT, sb_xTb, start=True, stop=True)
            nc.scalar.activation(actTb, ps_b, Ident, bias=sb_bl, scale=1.0)

            # ---- ZT[d,c] = sum_a tx[a,d] * mask[c,a];  bloZT = blo * ZT ----
            zt_ps = setps.tile([128, CB], FP, tag="set_mm")
            for t in range(T):
                nc.tensor.matmul(
                    zt_ps, tx_nat[:, t, :], sb_mTb[:, t, :],
                    start=(t == 0), stop=(t == T - 1),
                )
            bloZT = singles.tile([128, CB], FP)
            nc.vector.tensor_scalar_mul(bloZT, zt_ps, sb_blo)

            # ---- main loop over this core's 128 output rows, 4 at a time ----
            # software pipelined: iteration i issues masks(i), matmuls(i-1),
            # g4+matvecs(i-2) so no engine queue blocks on a later stage.
            y1t_ps = y1t_pool.tile([128, CB], FP)  # Y1^T columns, [d, c]
            d0, d1 = DVE_TSL
            g0, g1 = GP_TSL
            NQ = CB // QUAD
            ma_t = [None] * NQ
            s4_t = [None] * NQ

            def stage_masks(cq):
                c0 = cq * QUAD
                # ma[p, t, d, j] = tx[p, t, d] * m[p, t, c0+j]   ([t,d,c] order)
                ma = ma_pool.tile([128, T, 128, QUAD], BF, tag="ma")
                ma_t[cq] = ma
                nc.vector.tensor_mul(
                    ma[:, d0:d1, :, :],
                    tx_nat[:, d0:d1, :].unsqueeze(3).broadcast_to((128, d1 - d0, 128, QUAD)),
                    sb_mTb[:, d0:d1, c0:c0 + QUAD].unsqueeze(2).broadcast_to((128, d1 - d0, 128, QUAD)),
                )
                nc.gpsimd.tensor_mul(
                    ma[:, g0:g1, :, :],
                    tx_nat[:, g0:g1, :].unsqueeze(3).broadcast_to((128, g1 - g0, 128, QUAD)),
                    sb_mTb[:, g0:g1, c0:c0 + QUAD].unsqueeze(2).broadcast_to((128, g1 - g0, 128, QUAD)),
                )
                for t in SC_T:
                    for j in range(QUAD):
                        nc.scalar.mul(
                            ma[:, t, :, j], tx_nat[:, t, :], sb_mTf[:, t, c0 + j:c0 + j + 1]
                        )

            def stage_matmuls(cq):
                # S for the quad: 8 wide accumulating matmuls, out [e, (d, c)]
                s4 = s4_pool.tile([128, 128, QUAD], FP)
                s4_t[cq] = s4
                ma = ma_t[cq]
                for t in range(T):
                    nc.tensor.matmul(
                        s4[:, :, :], act_nat[:, t, :], ma[:, t, :, :],
                        start=(t == 0), stop=(t == T - 1),
                    )

            def stage_final(cq):
                c0 = cq * QUAD
                s4 = s4_t[cq]
                # g4[e, d, c] = S[e, d, c] * WloT[e, d]: one packed DVE op
                # straight out of PSUM; the matvec stationary reads stride-4.
                g4 = g_pool.tile([128, 128, QUAD], BF, tag="g4")
                nc.vector.tensor_mul(g4, s4, sb_Wlodc)
                for j in range(QUAD):
                    c = c0 + j
                    nc.tensor.matmul(
                        y1t_ps[:, c:c + 1], g4[:, :, j], actTb[:, c:c + 1],
                        start=True, stop=True,
                    )

            for i in range(NQ + 2):
                if i < NQ:
                    stage_masks(i)
                if 1 <= i < NQ + 1:
                    stage_matmuls(i - 1)
                if i >= 2:
                    stage_final(i - 2)

            # ---- combine, transpose back, residual, LayerNorm ----
            ident = singles.tile([128, 128], FP)
            make_identity(nc, ident)

            yt_sb = singles.tile([128, CB], FP)
            nc.vector.tensor_add(yt_sb, y1t_ps, bloZT)           # [d, c]
            y_ps = trps.tile([128, 128], FP, tag="tr")
            nc.tensor.transpose(y_ps, yt_sb, ident)              # [c, d]
            y_sb = singles.tile([CB, D], FP)
            nc.vector.tensor_add(y_sb, y_ps, sb_xrow)            # + x residual

            stats = singles.tile([CB, nc.vector.BN_STATS_DIM], FP)
            nc.vector.bn_stats(stats, y_sb)
            mv = singles.tile([CB, 2], FP)
            nc.vector.bn_aggr(mv, stats)
            nc.vector.tensor_scalar_sub(y_sb, y_sb, mv[:, 0:1])  # y - mean
            sd = singles.tile([CB, 1], FP)
            nc.scalar.activation(sd, mv[:, 1:2], Sqrt, bias=sb_eps, scale=1.0)
            rstd = singles.tile([CB, 1], FP)
            nc.vector.reciprocal(rstd, sd)
            nc.vector.tensor_scalar_mul(y_sb, y_sb, rstd)
            nc.vector.tensor_mul(y_sb, y_sb, sb_gam)
            nc.vector.tensor_add(y_sb, y_sb, sb_bet)

            nc.sync.dma_start(out[:, :], y_sb)

    return nc


_NC_CACHE = None


def _get_nc():
    global _NC_CACHE
    if _NC_CACHE is None:
        _NC_CACHE = build_nc()
        _NC_CACHE.finalize()
    return _NC_CACHE


def _prepare_in_maps(x, mask, Wl, bl, Wlo, blo, Wl2, bl2, gamma, beta):
    f32 = np.float32
    bf16 = ml_dtypes.bfloat16
    x0 = np.ascontiguousarray(np.asarray(x, f32)[0])          # [L, D]
    m = np.asarray(mask)[0].astype(f32)                       # [L, L] (c, a)
    xT = np.ascontiguousarray(x0.T)                           # [128, L]
    WlT = np.ascontiguousarray(np.asarray(Wl, f32).T)
    Wl2T = np.ascontiguousarray(np.asarray(Wl2, f32).T)
    WloT = np.ascontiguousarray(np.asarray(Wlo, f32).T)       # [e, d]
    Wlodc = np.ascontiguousarray(
        np.broadcast_to(WloT[:, :, None], (128, 128, QUAD))
    ).astype(bf16)
    bl_c = np.asarray(bl, f32).reshape(128, 1)
    blo_c = np.asarray(blo, f32).reshape(128, 1)
    blrow = np.asarray(bl, f32).reshape(1, 128).astype(bf16)
    bl2row = np.asarray(bl2, f32).reshape(1, 128).astype(bf16)
    gam_b = np.ascontiguousarray(np.broadcast_to(np.asarray(gamma, f32), (CB, D)))
    bet_b = np.ascontiguousarray(np.broadcast_to(np.asarray(beta, f32), (CB, D)))
    xT_bf = xT.astype(bf16)

    in_maps = []
    for k in range(NCORES):
        blk = slice(k * CB, (k + 1) * CB)
        mTk = m[blk, :].T.reshape(T, 128, CB).transpose(1, 0, 2)  # [p, t, c]
        mTk = np.ascontiguousarray(mTk)
        in_maps.append({
            "xT": xT_bf,
            "xTb": np.ascontiguousarray(xT_bf[:, blk]),
            "xrow": np.ascontiguousarray(x0[blk]),
            "mTb": mTk.astype(bf16),
            "mTf": mTk,
            "WlT": WlT.astype(bf16),
            "Wl2T": Wl2T.astype(bf16),
            "Wlodc": Wlodc,
            "blrow": blrow,
            "bl2row": bl2row,
            "bl": bl_c,
            "blo": blo_c,
            "gam": gam_b,
            "bet": bet_b,
        })
    return in_maps


def kernel(x, mask, Wl, bl, Wlo, blo, Wl2, bl2, gamma, beta):
    in_maps = _prepare_in_maps(x, mask, Wl, bl, Wlo, blo, Wl2, bl2, gamma, beta)
    res = run_bass_kernel_spmd(_get_nc(), in_maps, core_ids=list(range(NCORES)))
    y = np.concatenate([res.results[k]["out"] for k in range(NCORES)], axis=0)
    return y.reshape(B, L, D).astype(np.float32)
